# revision 10
# baseline (speedup 1.0000x reference)
"""Affinity-propagation (pixel-adaptive 3x3 conv, 16 iters) Trainium2 kernel.

Sharding: data-parallel over batch. B=8 batches -> 8 NeuronCores, one batch
per core. Each core runs the full 16-iteration propagation for its batch
entirely in SBUF.

Per-core layout: partition p (0..127) owns image rows 2p and 2p+1.
SBUF x-buffers are [128, r=4, c=CG, w=260]:
  r=0: halo row 2p-1, r=1: row 2p, r=2: row 2p+1, r=3: halo row 2p+2
  w: 2 zero pad cols each side, interior w in [2, 258)
With this layout every conv tap (di, dj) is a pure free-dim offset:
  out[:, 1:3, :, 2:258] += k'[di,dj] * in[:, 1+di:3+di, :, 2+dj:258+dj]
Halo rows are refreshed after each iteration by two cross-partition
SBUF->SBUF DMAs. The sparse-depth blend is folded into the weights:
  x_next = a + sum_taps k'_t (*) shift_t(x),  a = mask*x0, k' = (1-mask)*k.
Channels processed in groups of CG to fit SBUF.
"""

import numpy as np

import concourse.bass as bass
import concourse.mybir as mybir
from concourse.tile import TileContext
from concourse.bass_utils import run_bass_kernel_spmd

B, C, H, W = 8, 32, 256, 256
KG = 8          # guided channels (= 9 taps - zero center)
NCORES = 8
CG = 8          # channels per group
NG = C // CG
P = 128
R = 4
WP = 260        # padded width, interior [2, 258)
F32 = mybir.dt.float32
Alu = mybir.AluOpType
Act = mybir.ActivationFunctionType

# tap index -> (di, dj), skipping the zero center tap. Order matches the
# reference: kernel = concat(g[:4], 0, g[4:]) reshaped (3,3).
TAPS = [(-1, -1), (-1, 0), (-1, 1), (0, -1), (0, 1), (1, -1), (1, 0), (1, 1)]


def _legalize_waits(nc) -> None:
    """This container's walrus build rejects instructions with more than one
    semaphore wait ("Too many sync wait commands"). Split any multi-wait
    instruction: keep one wait on it, hoist the others onto single-wait
    Drain carrier instructions inserted immediately before it on the same
    engine (sequential waits == waiting on all)."""
    n = 0
    for fn in nc.m.functions:
        for b in fn.blocks:
            insts = b.instructions
            i = 0
            while i < len(insts):
                ins = insts[i]
                si = getattr(ins, "sync_info", None)
                if si is not None and si.on_wait and len(si.on_wait) > 1:
                    waits = list(si.on_wait)
                    for w in waits[:-1]:
                        carrier = mybir.InstDrain(
                            name=f"{ins.name}_lw{n}",
                            engine=ins.engine,
                            ins=[],
                            outs=[],
                            sync_info=mybir.SyncInfo(on_wait=[w], on_update=[]),
                        )
                        n += 1
                        nc.register_instruction(carrier, overwrite=True)
                        insts.insert(i, carrier)
                        i += 1
                    ins.sync_info = mybir.SyncInfo(
                        on_wait=[waits[-1]], on_update=list(si.on_update))
                i += 1


def build_program(prop_time: int) -> bass.Bass:
    nc = bass.Bass("TRN2", target_bir_lowering=False, debug=False,
                   num_devices=NCORES)
    x_d = nc.dram_tensor("x_in", [C, H, W], F32, kind="ExternalInput").ap()
    g_d = nc.dram_tensor("guided_in", [KG, H, W], F32, kind="ExternalInput").ap()
    s_d = nc.dram_tensor("sparse_in", [1, H, W], F32, kind="ExternalInput").ap()
    o_d = nc.dram_tensor("x_out", [C, H, W], F32, kind="ExternalOutput").ap()

    with TileContext(nc) as tc:
        with tc.tile_pool(name="pers", bufs=1) as pool:
            _body(nc, tc, pool, x_d, g_d, s_d, o_d, prop_time)
    _legalize_waits(nc)
    return nc


def _body(nc, tc, pool, x_d, g_d, s_d, o_d, prop_time):
    v = nc.vector

    # ---- persistent SBUF tiles (one pool, one slot per tag) ----
    xa = pool.tile([P, R, CG, WP], F32, name="xa")
    xb = pool.tile([P, R, CG, WP], F32, name="xb")
    a_t = pool.tile([P, 2, CG, W], F32, name="a_t")     # mask * x0 (interior)
    tmp0 = pool.tile([P, 2, CG, W], F32, name="tmp0")
    tmp1 = pool.tile([P, 2, CG, W], F32, name="tmp1")
    ktile = pool.tile([P, KG, 2, W], F32, name="ktile")  # (1-mask)*softmax wts
    gt = pool.tile([P, 2, KG, W], F32, name="gt")       # guided / exp workspace
    sp = pool.tile([P, 2, W], F32, name="sp")           # sparse depth rows
    mask = pool.tile([P, 2, W], F32, name="mask")
    s_w = pool.tile([P, 2, W], F32, name="s_w")         # (1-mask) / sum(exp)
    rsum = pool.tile([P, 2, W], F32, name="rsum")

    # DRAM views: row h = 2p + r
    xv = x_d.rearrange("c (p r) w -> p r c w", r=2)
    gv = g_d.rearrange("g (p r) w -> p r g w", r=2)
    sv = s_d[0].rearrange("(p r) w -> p r w", r=2)
    ov = o_d.rearrange("c (p r) w -> p r c w", r=2)

    # zero pads + halo edges once
    v.memset(xa[:], 0.0)
    v.memset(xb[:], 0.0)

    # ---- one-time weight setup ----
    nc.sync.dma_start(out=gt[:], in_=gv)
    nc.sync.dma_start(out=sp[:], in_=sv)
    # softmax over the 8 guided channels (no max-subtraction: inputs are
    # O(1) randn, exp stays well inside fp32 range)
    nc.scalar.activation(out=gt[:], in_=gt[:], func=Act.Exp)
    v.tensor_add(out=rsum[:], in0=gt[:, :, 0, :], in1=gt[:, :, 1, :])
    for g in range(2, KG):
        v.tensor_add(out=rsum[:], in0=rsum[:], in1=gt[:, :, g, :])
    v.reciprocal(out=rsum[:], in_=rsum[:])
    # mask = sparse > 0 ; s_w = (sparse <= 0) / sum(exp)
    v.tensor_scalar(out=mask[:], in0=sp[:], scalar1=0.0, scalar2=None,
                    op0=Alu.is_gt)
    v.tensor_scalar(out=s_w[:], in0=sp[:], scalar1=0.0, scalar2=None,
                    op0=Alu.is_le)
    v.tensor_mul(out=s_w[:], in0=s_w[:], in1=rsum[:])
    for g in range(KG):
        v.tensor_mul(out=ktile[:, g], in0=gt[:, :, g, :], in1=s_w[:])

    mask_b = mask[:].unsqueeze(2).broadcast_to([P, 2, CG, W])

    for grp in range(NG):
        c0 = grp * CG
    # ---- load this channel group (interior + both halo rows) ----
        xg = xv[:, :, c0:c0 + CG, :]
        nc.sync.dma_start(out=xa[:, 1:3, :, 2:258], in_=xg)
        nc.sync.dma_start(out=xa[1:128, 0:1, :, 2:258], in_=xg[0:127, 1:2])
        nc.sync.dma_start(out=xa[0:127, 3:4, :, 2:258], in_=xg[1:128, 0:1])
        # a = mask * x0
        v.tensor_mul(out=a_t[:], in0=xa[:, 1:3, :, 2:258], in1=mask_b)

        src, dst = xa, xb
        for _ in range(prop_time):
            acc = dst[:, 1:3, :, 2:258]
            for ti, (di, dj) in enumerate(TAPS):
                inp = src[:, 1 + di:3 + di, :, 2 + dj:258 + dj]
                kb = (ktile[:, ti:ti + 1].transpose([0, 2, 1, 3])
                      .broadcast_to([P, 2, CG, W]))
                tmp = tmp0 if ti % 2 == 0 else tmp1
                v.tensor_mul(out=tmp[:], in0=inp, in1=kb)
                if ti == 0:
                    v.tensor_add(out=acc, in0=tmp[:], in1=a_t[:])
                else:
                    v.tensor_add(out=acc, in0=acc, in1=tmp[:])
            # refresh halo rows of the freshly written buffer
            nc.sync.dma_start(out=dst[1:128, 0:1], in_=dst[0:127, 2:3])
            nc.sync.dma_start(out=dst[0:127, 3:4], in_=dst[1:128, 1:2])
            src, dst = dst, src

        nc.sync.dma_start(out=ov[:, :, c0:c0 + CG, :],
                          in_=src[:, 1:3, :, 2:258])


def _jit_sharded(nc, n_cores):
    """Build a jitted shard_map executable for `nc` (no donation so device
    buffers can be reused across timing runs). Returns (fn, in_names,
    out_names, out_avals, n_params)."""
    import jax
    from jax.sharding import Mesh, PartitionSpec
    from jax.experimental.shard_map import shard_map
    from concourse import bass2jax

    bass2jax.install_neuronx_cc_hook()
    partition_name = (nc.partition_id_tensor.name
                      if nc.partition_id_tensor else None)
    in_names, out_names, out_avals = [], [], []
    for alloc in nc.m.functions[0].allocations:
        if not isinstance(alloc, mybir.MemoryLocationSet):
            continue
        name = alloc.memorylocations[0].name
        if alloc.kind == "ExternalInput":
            if name != partition_name:
                in_names.append(name)
        elif alloc.kind == "ExternalOutput":
            out_names.append(name)
            out_avals.append(jax.core.ShapedArray(
                tuple(alloc.tensor_shape), mybir.dt.np(alloc.dtype)))
    n_params = len(in_names)
    in_names = in_names + out_names
    if partition_name is not None:
        in_names.append(partition_name)

    def _fn(*args):
        operands = list(args)
        if partition_name is not None:
            operands.append(bass2jax.partition_id_tensor())
        return tuple(bass2jax._bass_exec_p.bind(
            *operands, out_avals=tuple(out_avals), in_names=tuple(in_names),
            out_names=tuple(out_names), lowering_input_output_aliases=(),
            sim_require_finite=True, sim_require_nnan=True, nc=nc))

    devices = jax.devices()[:n_cores]
    mesh = Mesh(np.asarray(devices), ("core",))
    nin = n_params + len(out_names)
    fn = jax.jit(shard_map(_fn, mesh=mesh,
                           in_specs=(PartitionSpec("core"),) * nin,
                           out_specs=(PartitionSpec("core"),) * len(out_names),
                           check_rep=False), keep_unused=True)
    return fn, in_names, out_names, out_avals, n_params


def _time_program(nc, in_maps, n_cores, iters):
    import jax
    import time
    fn, in_names, out_names, out_avals, n_params = _jit_sharded(nc, n_cores)
    concat = [np.concatenate([np.asarray(m[in_names[i]])[None] for m in in_maps])
              .reshape(n_cores * in_maps[0][in_names[i]].shape[0],
                       *in_maps[0][in_names[i]].shape[1:])
              for i in range(n_params)]
    zeros = [np.zeros((n_cores * a.shape[0], *a.shape[1:]), a.dtype)
             for a in out_avals]
    dev_in = [jax.device_put(a) for a in concat + zeros]
    out = fn(*dev_in)  # compile + warmup
    jax.block_until_ready(out)
    times = []
    for _ in range(iters):
        t0 = time.perf_counter()
        out = fn(*dev_in)
        jax.block_until_ready(out)
        times.append(time.perf_counter() - t0)
    return min(times) * 1e9, out, out_names, out_avals


def _null_program():
    nc = bass.Bass("TRN2", target_bir_lowering=False, debug=False,
                   num_devices=NCORES)
    i_d = nc.dram_tensor("nul_in", [1, 16], F32, kind="ExternalInput").ap()
    o_d = nc.dram_tensor("nul_out", [1, 16], F32, kind="ExternalOutput").ap()
    with TileContext(nc) as tc:
        with tc.tile_pool(name="p", bufs=1) as pool:
            t = pool.tile([1, 16], F32, name="t")
            nc.sync.dma_start(out=t[:], in_=i_d)
            nc.sync.dma_start(out=o_d, in_=t[:])
    _legalize_waits(nc)
    return nc


def timed_run(inputs, iters=20):
    """Return best-effort HW exec time (ns) for the full 8-core kernel,
    with axon dispatch overhead measured via a null program and subtracted."""
    x = np.asarray(inputs["x"], dtype=np.float32)
    guided = np.asarray(inputs["guided"], dtype=np.float32)
    sparse = np.asarray(inputs["sparse_depth"], dtype=np.float32)
    prop_time = int(np.asarray(inputs["prop_time"]))
    nc = build_program(prop_time)
    in_maps = [{"x_in": x[b], "guided_in": guided[b], "sparse_in": sparse[b]}
               for b in range(B)]
    total_ns, _, _, _ = _time_program(nc, in_maps, NCORES, iters)
    null_maps = [{"nul_in": np.zeros((1, 16), np.float32)} for _ in range(B)]
    null_ns, _, _, _ = _time_program(_null_program(), null_maps, NCORES, iters)
    print(f"  total roundtrip: {total_ns:.0f} ns, null roundtrip: {null_ns:.0f} ns")
    return total_ns - null_ns


def kernel(**inputs) -> np.ndarray:
    x = np.ascontiguousarray(np.asarray(inputs["x"], dtype=np.float32))
    guided = np.ascontiguousarray(np.asarray(inputs["guided"], dtype=np.float32))
    sparse = np.ascontiguousarray(np.asarray(inputs["sparse_depth"],
                                             dtype=np.float32))
    prop_time = int(np.asarray(inputs["prop_time"]))
    assert x.shape == (B, C, H, W), x.shape

    nc = build_program(prop_time)
    in_maps = [
        {"x_in": x[b], "guided_in": guided[b], "sparse_in": sparse[b]}
        for b in range(B)
    ]
    res = run_bass_kernel_spmd(nc, in_maps, core_ids=list(range(NCORES)))
    return np.stack([res.results[b]["x_out"] for b in range(B)], axis=0)
